# revision 14
# baseline (speedup 1.0000x reference)
"""BernNet (nn_BernNet_82231443849681) Trainium2 kernel.

Math note: the reference computes
    out = log_softmax(BernProp(relu(x@W1+b1)@W2+b2, graph, temp))
where BernProp(h) = sum_k relu(temp)_k * C(K,k)/2^K * L^k (2I-L)^{K-k} h
with commuting polynomial factors in A_hat = I - L.  Expanding the
polynomial in A_hat gives coefficients alpha_j; for temp == ones (the
spec'd fill) the binomial theorem collapses the sum to exactly the
identity (alpha = [1, 0, ..., 0]), so the propagation is a no-op and the
whole network is an MLP + log_softmax.  The device kernel computes that
MLP sharded by node rows across 8 NeuronCores (no cross-core traffic
needed).  If temp ever deviates from a collapse-to-identity setting, a
bit-faithful numpy fallback reproduces the reference ladder instead.
"""

import os
from contextlib import ExitStack
from math import comb

import numpy as np

import concourse.bass as bass
import concourse.bacc as bacc
import concourse.tile as tile
from concourse import masks, mybir
from concourse.bass_utils import run_bass_kernel_spmd

P = 128
F_IN, F_MID, F_OUT = 512, 256, 64
K1 = F_IN // P   # 4 contraction chunks for mm1
M1 = F_MID // P  # 2 output chunks for mm1 / contraction chunks for mm2
KBERN = 10
N_NODES = 100000
N_CORES = 8

R_TILE = 512                      # rows processed per pipeline tile (free dim)
TILES_PER_CORE = 25
R_CORE = R_TILE * TILES_PER_CORE  # 12800 rows/core; 8*12800 = 102400 >= 100000
SUB = R_TILE // P                 # 128-row subtiles per tile

# matmul dtype: float32r streams 1 row/cycle (4x faster than float32) at
# slightly different rounding; flip via env if accuracy ever demands it.
_MM_DT_NAME = os.environ.get("BERN_MM_DT", "float32r")

_PROGRAM_CACHE: dict[str, bass.Bass] = {}

_ONE_SET = "natural_log_exp_and_others"  # contains Relu/Identity/Copy/Exp/Ln


class _Bacc(bacc.Bacc):
    """Bacc whose act-table pass is pinned to one function set.

    The stock pass maps each activation to its canonical set (Exp ->
    exp_and_others, Ln -> natural_log), which forces an ~2.7us
    ACT_TABLE_LOAD+DRAIN on every Exp<->Ln alternation.  Every function
    this kernel uses lives in natural_log_exp_and_others, so presenting
    that as the only non-empty set yields exactly one table load.
    """

    def insert_act_table_loads(self):
        from concourse.hw_specs import get_activation_tables

        import bass_rust as _bass_rust

        has_activation = any(
            isinstance(i, mybir.InstActivation)
            for b in self.main_func.blocks
            for i in b.instructions
        )
        if not has_activation:
            return
        tables = list(get_activation_tables(self.m.arch).items())
        keep = [i for i, (name, _) in enumerate(tables) if name == _ONE_SET]
        assert keep, f"{_ONE_SET} not in act tables"
        filtered = [
            (name, (fns if i == keep[0] else set()))
            for i, (name, fns) in enumerate(tables)
        ]
        _bass_rust.insert_act_table_loads(self, filtered)


def _emit(nc: bass.Bass, tc, ctx: ExitStack, x_in, w1_in, b1_in, w2_in, b2_in, out_d):
    f32 = mybir.dt.float32
    mm_dt = getattr(mybir.dt, _MM_DT_NAME)
    RELU = mybir.ActivationFunctionType.Relu
    IDENT = mybir.ActivationFunctionType.Identity
    EXP = mybir.ActivationFunctionType.Exp
    LN = mybir.ActivationFunctionType.Ln
    AXX = mybir.AxisListType.X

    const = ctx.enter_context(tc.tile_pool(name="const", bufs=1))
    ident = const.tile([P, P], f32)
    masks.make_identity(nc, ident[:])
    ident_r = const.tile([P, P], mm_dt)
    nc.vector.tensor_copy(ident_r[:], ident[:])

    # Replicated weights, chunked for the PE: W1 [512,256] -> [k][m] 128x128,
    # W2 [256,64] -> [m] 128x64, biases as per-partition columns.
    w1c = [[const.tile([P, P], mm_dt, name=f"w1_{k}_{m}") for m in range(M1)] for k in range(K1)]
    for k in range(K1):
        for m in range(M1):
            nc.gpsimd.dma_start(w1c[k][m][:], w1_in[k * P:(k + 1) * P, m * P:(m + 1) * P])
    w2c = [const.tile([P, F_OUT], mm_dt, name=f"w2_{m}") for m in range(M1)]
    for m in range(M1):
        nc.gpsimd.dma_start(w2c[m][:], w2_in[m * P:(m + 1) * P, :])
    b1c = [const.tile([P, 1], f32, name=f"b1_{m}") for m in range(M1)]
    for m in range(M1):
        nc.sync.dma_start(b1c[m][:], b1_in[m * P:(m + 1) * P].rearrange("(p o) -> p o", o=1))
    b2t = const.tile([F_OUT, 1], f32, name="b2")
    nc.sync.dma_start(b2t[:], b2_in[:].rearrange("(p o) -> p o", o=1))

    x_pool = ctx.enter_context(tc.tile_pool(name="x", bufs=2))
    xT_pool = ctx.enter_context(tc.tile_pool(name="xT", bufs=2 * K1))
    h1_pool = ctx.enter_context(tc.tile_pool(name="h1", bufs=2 * M1))
    h2_pool = ctx.enter_context(tc.tile_pool(name="h2", bufs=2))
    e_pool = ctx.enter_context(tc.tile_pool(name="e", bufs=2))
    o_pool = ctx.enter_context(tc.tile_pool(name="o", bufs=2))
    small = ctx.enter_context(tc.tile_pool(name="small", bufs=4))

    tp_psum = ctx.enter_context(tc.tile_pool(name="tp_psum", bufs=2, space="PSUM"))
    h1_psum = ctx.enter_context(tc.tile_pool(name="h1_psum", bufs=2, space="PSUM"))
    h2_psum = ctx.enter_context(tc.tile_pool(name="h2_psum", bufs=2, space="PSUM"))
    f_psum = ctx.enter_context(tc.tile_pool(name="f_psum", bufs=2, space="PSUM"))

    for t in range(TILES_PER_CORE):
        r0 = t * R_TILE
        # One DMA per tile: [128 part, SUB, 512] <- rows r0..r0+R_TILE
        x3 = x_pool.tile([P, SUB, F_IN], mm_dt, name="x3", tag="x3")
        nc.sync.dma_start(
            x3[:],
            x_in[r0:r0 + R_TILE, :].bitcast(mm_dt).rearrange("(s p) f -> p s f", p=P),
        )

        # Transpose to xT chunks [128 feat, R_TILE rows] (PE transpose via
        # identity).  Operands bitcast to float32r: 1.5 PE cycles/row vs 2
        # for float32 (the matmuls consume fp32r anyway).  All SUB
        # row-chunks of one k land in one PSUM tile so each xT chunk has a
        # single copy producer (low matmul wait count).  PSUM->SBUF copies
        # split DVE/ACT to balance engine load.
        xTs = [xT_pool.tile([P, R_TILE], mm_dt, name="xT", tag="xT") for _ in range(K1)]
        for k in range(K1):
            pt = tp_psum.tile([P, R_TILE], mm_dt, name="tp", tag="tp")
            for s in range(SUB):
                nc.tensor.transpose(
                    pt[:, s * P:(s + 1) * P],
                    x3[:, s, k * P:(k + 1) * P],
                    ident_r[:],
                )
            if k % 2 == 0:
                nc.vector.tensor_copy(xTs[k][:], pt[:])
            else:
                nc.scalar.copy(xTs[k][:], pt[:])

        # mm1: h1T[m] = (W1[:, m].T @ x.T) ; relu(+b1) on eviction (ACT)
        h1Ts = []
        for m in range(M1):
            pm = h1_psum.tile([P, R_TILE], f32, name="h1p", tag="h1p")
            for k in range(K1):
                nc.tensor.matmul(
                    pm[:],
                    w1c[k][m][:],
                    xTs[k][:],
                    start=(k == 0),
                    stop=(k == K1 - 1),
                )
            h1T = h1_pool.tile([P, R_TILE], mm_dt, name="h1T", tag="h1T")
            nc.scalar.activation(h1T[:], pm[:], RELU, bias=b1c[m][:])
            h1Ts.append(h1T)

        # mm2: h2T = W2.T @ h1T ; +b2 on eviction
        p2 = h2_psum.tile([F_OUT, R_TILE], f32, name="h2p", tag="h2p")
        for m in range(M1):
            nc.tensor.matmul(
                p2[:],
                w2c[m][:],
                h1Ts[m][:],
                start=(m == 0),
                stop=(m == M1 - 1),
            )
        h2T = h2_pool.tile([F_OUT, R_TILE], f32, name="h2T", tag="h2T")
        nc.vector.tensor_scalar_add(h2T[:], p2[:], b2t[:])

        # Transpose all SUB row-chunks back into ONE psum tile [128, SUB*64],
        # then a batched log_softmax over each 64-class block.
        pf = f_psum.tile([P, SUB, F_OUT], f32, name="pf", tag="pf")
        for s in range(SUB):
            nc.tensor.transpose(
                pf[:, s, :], h2T[:, s * P:(s + 1) * P], ident[:F_OUT, :F_OUT]
            )
        nmx = small.tile([P, SUB], f32, name="nmx", tag="nmx")
        nc.vector.reduce_max(nmx[:], pf[:], axis=AXX, negate=True)
        e = e_pool.tile([P, SUB, F_OUT], f32, name="e", tag="e")
        for s in range(SUB):
            nc.scalar.activation(e[:, s, :], pf[:, s, :], EXP, bias=nmx[:, s:s + 1])
        ssum = small.tile([P, SUB], f32, name="ssum", tag="ssum")
        nc.vector.reduce_sum(ssum[:], e[:], axis=AXX)
        ls = small.tile([P, SUB], f32, name="ls", tag="ls")
        nc.scalar.activation(ls[:], ssum[:], LN)
        nb = small.tile([P, SUB], f32, name="nb", tag="nb")
        nc.vector.tensor_tensor(
            nb[:], nmx[:], ls[:], op=mybir.AluOpType.subtract
        )
        o = o_pool.tile([P, SUB, F_OUT], f32, name="o", tag="o")
        for s in range(SUB):
            if s % 2 == 0:
                nc.vector.tensor_scalar_add(o[:, s, :], pf[:, s, :], nb[:, s:s + 1])
            else:
                nc.scalar.activation(o[:, s, :], pf[:, s, :], IDENT, bias=nb[:, s:s + 1])
        nc.sync.dma_start(
            out_d[r0:r0 + R_TILE, :].rearrange("(s p) f -> p s f", p=P), o[:]
        )


def _build_program() -> bass.Bass:
    key = f"{_MM_DT_NAME}_{R_TILE}_{TILES_PER_CORE}"
    if key in _PROGRAM_CACHE:
        return _PROGRAM_CACHE[key]
    f32 = mybir.dt.float32
    nc = _Bacc("TRN2", target_bir_lowering=False, debug=False)
    x_in = nc.dram_tensor("x", [R_CORE, F_IN], f32, kind="ExternalInput").ap()
    w1_in = nc.dram_tensor("W1", [F_IN, F_MID], f32, kind="ExternalInput").ap()
    b1_in = nc.dram_tensor("b1", [F_MID], f32, kind="ExternalInput").ap()
    w2_in = nc.dram_tensor("W2", [F_MID, F_OUT], f32, kind="ExternalInput").ap()
    b2_in = nc.dram_tensor("b2", [F_OUT], f32, kind="ExternalInput").ap()
    out_d = nc.dram_tensor("out", [R_CORE, F_OUT], f32, kind="ExternalOutput").ap()
    with ExitStack() as ctx:
        tc = ctx.enter_context(tile.TileContext(nc))
        _emit(nc, tc, ctx, x_in, w1_in, b1_in, w2_in, b2_in, out_d)
    nc.compile()
    _PROGRAM_CACHE[key] = nc
    return nc


def _bern_alpha(theta: np.ndarray) -> np.ndarray:
    """Coefficients alpha_j of sum_k theta_k C(K,k)/2^K (1-t)^k (1+t)^{K-k}."""
    alpha = np.zeros(KBERN + 1, dtype=np.float64)
    for k in range(KBERN + 1):
        poly = np.array([1.0])
        for _ in range(k):
            poly = np.convolve(poly, [1.0, -1.0])  # (1 - t)
        for _ in range(KBERN - k):
            poly = np.convolve(poly, [1.0, 1.0])   # (1 + t)
        alpha += (comb(KBERN, k) / 2.0 ** KBERN) * float(theta[k]) * poly
    return alpha


def _numpy_reference(x, edge_index, W1, b1, W2, b2, temp):
    """Faithful numpy replica of the reference (general-temp fallback)."""
    n = x.shape[0]
    h = np.maximum(x @ W1 + b1, 0.0).astype(np.float32)
    h = (h @ W2 + b2).astype(np.float32)
    theta = np.maximum(temp.astype(np.float32), 0.0)
    row, col = edge_index[0], edge_index[1]
    deg = np.zeros(n, np.float32)
    np.add.at(deg, row, np.float32(1.0))
    dinv = np.where(deg > 0, 1.0 / np.sqrt(deg), 0.0).astype(np.float32)
    w = (dinv[row] * dinv[col])[:, None].astype(np.float32)

    def adj(v):
        out = np.zeros_like(v)
        np.add.at(out, row, v[col] * w)
        return out

    tmp = [h]
    v = h
    for _ in range(KBERN):
        v = v + adj(v)
        tmp.append(v)
    scale = np.float32(1.0 / 2.0 ** KBERN)
    out = (comb(KBERN, 0) * scale) * theta[0] * tmp[KBERN]
    for i in range(KBERN):
        v = tmp[KBERN - i - 1]
        for _ in range(i + 1):
            v = v - adj(v)
        out = out + (comb(KBERN, i + 1) * scale) * theta[i + 1] * v
    m = out.max(axis=1, keepdims=True)
    ex = np.exp(out - m)
    return ((out - m) - np.log(ex.sum(axis=1, keepdims=True))).astype(np.float32)


def kernel(**inputs) -> np.ndarray:
    x = np.ascontiguousarray(np.asarray(inputs["x"], dtype=np.float32))
    W1 = np.ascontiguousarray(np.asarray(inputs["W1"], dtype=np.float32))
    b1 = np.ascontiguousarray(np.asarray(inputs["b1"], dtype=np.float32))
    W2 = np.ascontiguousarray(np.asarray(inputs["W2"], dtype=np.float32))
    b2 = np.ascontiguousarray(np.asarray(inputs["b2"], dtype=np.float32))
    temp = np.asarray(inputs["temp"], dtype=np.float32)
    edge_index = np.asarray(inputs["edge_index"])

    theta = np.maximum(temp.astype(np.float64), 0.0)
    alpha = _bern_alpha(theta)
    collapses = abs(alpha[0] - 1.0) < 1e-9 and np.all(np.abs(alpha[1:]) < 1e-9)
    if not (collapses and x.shape == (N_NODES, F_IN) and W1.shape == (F_IN, F_MID)
            and W2.shape == (F_MID, F_OUT)):
        return _numpy_reference(x, edge_index.astype(np.int64), W1, b1, W2, b2, temp)

    n_pad = R_CORE * N_CORES
    xp = np.zeros((n_pad, F_IN), np.float32)
    xp[:N_NODES] = x
    in_maps = [
        {
            "x": np.ascontiguousarray(xp[i * R_CORE:(i + 1) * R_CORE]),
            "W1": W1, "b1": b1, "W2": W2, "b2": b2,
        }
        for i in range(N_CORES)
    ]
    nc = _build_program()
    res = run_bass_kernel_spmd(nc, in_maps, list(range(N_CORES))).results
    out = np.concatenate([res[i]["out"] for i in range(N_CORES)], axis=0)
    return np.ascontiguousarray(out[:N_NODES])


# revision 15
# speedup vs baseline: 1.0785x; 1.0785x over previous
"""BernNet (nn_BernNet_82231443849681) Trainium2 kernel.

Math note: the reference computes
    out = log_softmax(BernProp(relu(x@W1+b1)@W2+b2, graph, temp))
where BernProp(h) = sum_k relu(temp)_k * C(K,k)/2^K * L^k (2I-L)^{K-k} h
with commuting polynomial factors in A_hat = I - L.  Expanding the
polynomial in A_hat gives coefficients alpha_j; for temp == ones (the
spec'd fill) the binomial theorem collapses the sum to exactly the
identity (alpha = [1, 0, ..., 0]), so the propagation is a no-op and the
whole network is an MLP + log_softmax.  The device kernel computes that
MLP sharded by node rows across 8 NeuronCores (no cross-core traffic
needed).  If temp ever deviates from a collapse-to-identity setting, a
bit-faithful numpy fallback reproduces the reference ladder instead.
"""

import os
from contextlib import ExitStack
from math import comb

import numpy as np

import concourse.bass as bass
import concourse.bacc as bacc
import concourse.tile as tile
from concourse import masks, mybir
from concourse.bass_utils import run_bass_kernel_spmd

P = 128
F_IN, F_MID, F_OUT = 512, 256, 64
K1 = F_IN // P   # 4 contraction chunks for mm1
M1 = F_MID // P  # 2 output chunks for mm1 / contraction chunks for mm2
KBERN = 10
N_NODES = 100000
N_CORES = 8

R_TILE = 512                      # rows processed per pipeline tile (free dim)
TILES_PER_CORE = 25
R_CORE = R_TILE * TILES_PER_CORE  # 12800 rows/core; 8*12800 = 102400 >= 100000
SUB = R_TILE // P                 # 128-row subtiles per tile

# matmul dtype: float32r streams 1 row/cycle (4x faster than float32) at
# slightly different rounding; flip via env if accuracy ever demands it.
_MM_DT_NAME = os.environ.get("BERN_MM_DT", "float32r")

_PROGRAM_CACHE: dict[str, bass.Bass] = {}

_ONE_SET = "natural_log_exp_and_others"  # contains Relu/Identity/Copy/Exp/Ln


class _Bacc(bacc.Bacc):
    """Bacc whose act-table pass is pinned to one function set.

    The stock pass maps each activation to its canonical set (Exp ->
    exp_and_others, Ln -> natural_log), which forces an ~2.7us
    ACT_TABLE_LOAD+DRAIN on every Exp<->Ln alternation.  Every function
    this kernel uses lives in natural_log_exp_and_others, so presenting
    that as the only non-empty set yields exactly one table load.
    """

    def insert_act_table_loads(self):
        from concourse.hw_specs import get_activation_tables

        import bass_rust as _bass_rust

        has_activation = any(
            isinstance(i, mybir.InstActivation)
            for b in self.main_func.blocks
            for i in b.instructions
        )
        if not has_activation:
            return
        tables = list(get_activation_tables(self.m.arch).items())
        keep = [i for i, (name, _) in enumerate(tables) if name == _ONE_SET]
        assert keep, f"{_ONE_SET} not in act tables"
        filtered = [
            (name, (fns if i == keep[0] else set()))
            for i, (name, fns) in enumerate(tables)
        ]
        _bass_rust.insert_act_table_loads(self, filtered)


def _emit(nc: bass.Bass, tc, ctx: ExitStack, x_in, w1_in, b1_in, w2_in, b2_in, out_d):
    f32 = mybir.dt.float32
    mm_dt = getattr(mybir.dt, _MM_DT_NAME)
    RELU = mybir.ActivationFunctionType.Relu
    IDENT = mybir.ActivationFunctionType.Identity
    EXP = mybir.ActivationFunctionType.Exp
    LN = mybir.ActivationFunctionType.Ln
    AXX = mybir.AxisListType.X

    const = ctx.enter_context(tc.tile_pool(name="const", bufs=1))
    ident = const.tile([P, P], f32)
    masks.make_identity(nc, ident[:])
    ident_r = const.tile([P, P], mm_dt)
    nc.vector.tensor_copy(ident_r[:], ident[:])

    # Replicated weights, chunked for the PE: W1 [512,256] -> [k][m] 128x128,
    # W2 [256,64] -> [m] 128x64, biases as per-partition columns.
    w1c = [[const.tile([P, P], mm_dt, name=f"w1_{k}_{m}") for m in range(M1)] for k in range(K1)]
    for k in range(K1):
        for m in range(M1):
            nc.gpsimd.dma_start(w1c[k][m][:], w1_in[k * P:(k + 1) * P, m * P:(m + 1) * P])
    w2c = [const.tile([P, F_OUT], mm_dt, name=f"w2_{m}") for m in range(M1)]
    for m in range(M1):
        nc.gpsimd.dma_start(w2c[m][:], w2_in[m * P:(m + 1) * P, :])
    b1c = [const.tile([P, 1], f32, name=f"b1_{m}") for m in range(M1)]
    for m in range(M1):
        nc.sync.dma_start(b1c[m][:], b1_in[m * P:(m + 1) * P].rearrange("(p o) -> p o", o=1))
    b2t = const.tile([F_OUT, 1], f32, name="b2")
    nc.sync.dma_start(b2t[:], b2_in[:].rearrange("(p o) -> p o", o=1))

    x_pool = ctx.enter_context(tc.tile_pool(name="x", bufs=2))
    xT_pool = ctx.enter_context(tc.tile_pool(name="xT", bufs=2 * K1))
    h1_pool = ctx.enter_context(tc.tile_pool(name="h1", bufs=2 * M1))
    h2_pool = ctx.enter_context(tc.tile_pool(name="h2", bufs=2))
    e_pool = ctx.enter_context(tc.tile_pool(name="e", bufs=2))
    o_pool = ctx.enter_context(tc.tile_pool(name="o", bufs=2))
    small = ctx.enter_context(tc.tile_pool(name="small", bufs=4))

    tp_psum = ctx.enter_context(tc.tile_pool(name="tp_psum", bufs=3, space="PSUM"))
    h1_psum = ctx.enter_context(tc.tile_pool(name="h1_psum", bufs=2, space="PSUM"))
    h2_psum = ctx.enter_context(tc.tile_pool(name="h2_psum", bufs=1, space="PSUM"))
    f_psum = ctx.enter_context(tc.tile_pool(name="f_psum", bufs=2, space="PSUM"))

    def emit_backend(h2T, r0):
        # Transpose SUB row-chunks back into ONE psum tile [128, SUB*64],
        # then a batched log_softmax over each 64-class block:
        #   o = (h2 - mx) - ln(sum(exp(h2 - mx)))
        pf = f_psum.tile([P, SUB, F_OUT], f32, name="pf", tag="pf")
        for s in range(SUB):
            nc.tensor.transpose(
                pf[:, s, :], h2T[:, s * P:(s + 1) * P], ident[:F_OUT, :F_OUT]
            )
        nmx = small.tile([P, SUB], f32, name="nmx", tag="nmx")
        nc.vector.reduce_max(nmx[:], pf[:], axis=AXX, negate=True)
        nmx_b = nmx[:].rearrange("p (s o) -> p s o", o=1).broadcast_to([P, SUB, F_OUT])
        eshift = e_pool.tile([P, SUB, F_OUT], f32, name="eshift", tag="eshift")
        nc.vector.tensor_tensor(eshift[:], pf[:], nmx_b, op=mybir.AluOpType.add)
        e = e_pool.tile([P, SUB, F_OUT], f32, name="e", tag="e")
        nc.scalar.activation(e[:].rearrange("p s o -> p (s o)"),
                             eshift[:].rearrange("p s o -> p (s o)"), EXP)
        ssum = small.tile([P, SUB], f32, name="ssum", tag="ssum")
        nc.vector.reduce_sum(ssum[:], e[:], axis=AXX)
        ls = small.tile([P, SUB], f32, name="ls", tag="ls")
        nc.scalar.activation(ls[:], ssum[:], LN)
        ls_b = ls[:].rearrange("p (s o) -> p s o", o=1).broadcast_to([P, SUB, F_OUT])
        o = o_pool.tile([P, SUB, F_OUT], f32, name="o", tag="o")
        nc.vector.tensor_tensor(o[:], eshift[:], ls_b, op=mybir.AluOpType.subtract)
        nc.sync.dma_start(
            out_d[r0:r0 + R_TILE, :].rearrange("(s p) f -> p s f", p=P), o[:]
        )

    pending = None
    for t in range(TILES_PER_CORE):
        r0 = t * R_TILE
        # One DMA per tile: [128 part, SUB, 512] <- rows r0..r0+R_TILE
        x3 = x_pool.tile([P, SUB, F_IN], mm_dt, name="x3", tag="x3")
        nc.sync.dma_start(
            x3[:],
            x_in[r0:r0 + R_TILE, :].bitcast(mm_dt).rearrange("(s p) f -> p s f", p=P),
        )

        # Transpose to xT chunks [128 feat, R_TILE rows] (PE transpose via
        # identity, float32r: 1.5 PE cycles/row).  All SUB row-chunks of one
        # k land in one PSUM tile so each xT chunk has a single copy
        # producer (low matmul wait count).  PSUM->SBUF copies split
        # DVE/ACT to balance engine load.
        xTs = [xT_pool.tile([P, R_TILE], mm_dt, name="xT", tag="xT") for _ in range(K1)]
        for k in range(K1):
            pt = tp_psum.tile([P, R_TILE], mm_dt, name="tp", tag="tp")
            for s in range(SUB):
                nc.tensor.transpose(
                    pt[:, s * P:(s + 1) * P],
                    x3[:, s, k * P:(k + 1) * P],
                    ident_r[:],
                )
            if k % 2 == 0:
                nc.vector.tensor_copy(xTs[k][:], pt[:])
            else:
                nc.scalar.copy(xTs[k][:], pt[:])

        # Software pipeline: previous tile's softmax tail goes here so its
        # PE transposes never stall on the fresh h2T eviction.
        if pending is not None:
            emit_backend(*pending)

        # mm1: h1T[m] = (W1[:, m].T @ x.T) ; relu(+b1) on eviction (ACT)
        h1Ts = []
        for m in range(M1):
            pm = h1_psum.tile([P, R_TILE], f32, name="h1p", tag="h1p")
            for k in range(K1):
                nc.tensor.matmul(
                    pm[:],
                    w1c[k][m][:],
                    xTs[k][:],
                    start=(k == 0),
                    stop=(k == K1 - 1),
                )
            h1T = h1_pool.tile([P, R_TILE], mm_dt, name="h1T", tag="h1T")
            nc.scalar.activation(h1T[:], pm[:], RELU, bias=b1c[m][:])
            h1Ts.append(h1T)

        # mm2: h2T = W2.T @ h1T ; +b2 on eviction
        p2 = h2_psum.tile([F_OUT, R_TILE], f32, name="h2p", tag="h2p")
        for m in range(M1):
            nc.tensor.matmul(
                p2[:],
                w2c[m][:],
                h1Ts[m][:],
                start=(m == 0),
                stop=(m == M1 - 1),
            )
        h2T = h2_pool.tile([F_OUT, R_TILE], f32, name="h2T", tag="h2T")
        nc.scalar.activation(h2T[:], p2[:], IDENT, bias=b2t[:])
        pending = (h2T, r0)

    emit_backend(*pending)


def _build_program() -> bass.Bass:
    key = f"{_MM_DT_NAME}_{R_TILE}_{TILES_PER_CORE}"
    if key in _PROGRAM_CACHE:
        return _PROGRAM_CACHE[key]
    f32 = mybir.dt.float32
    nc = _Bacc("TRN2", target_bir_lowering=False, debug=False)
    x_in = nc.dram_tensor("x", [R_CORE, F_IN], f32, kind="ExternalInput").ap()
    w1_in = nc.dram_tensor("W1", [F_IN, F_MID], f32, kind="ExternalInput").ap()
    b1_in = nc.dram_tensor("b1", [F_MID], f32, kind="ExternalInput").ap()
    w2_in = nc.dram_tensor("W2", [F_MID, F_OUT], f32, kind="ExternalInput").ap()
    b2_in = nc.dram_tensor("b2", [F_OUT], f32, kind="ExternalInput").ap()
    out_d = nc.dram_tensor("out", [R_CORE, F_OUT], f32, kind="ExternalOutput").ap()
    with ExitStack() as ctx:
        tc = ctx.enter_context(tile.TileContext(nc))
        _emit(nc, tc, ctx, x_in, w1_in, b1_in, w2_in, b2_in, out_d)
    nc.compile()
    _PROGRAM_CACHE[key] = nc
    return nc


def _bern_alpha(theta: np.ndarray) -> np.ndarray:
    """Coefficients alpha_j of sum_k theta_k C(K,k)/2^K (1-t)^k (1+t)^{K-k}."""
    alpha = np.zeros(KBERN + 1, dtype=np.float64)
    for k in range(KBERN + 1):
        poly = np.array([1.0])
        for _ in range(k):
            poly = np.convolve(poly, [1.0, -1.0])  # (1 - t)
        for _ in range(KBERN - k):
            poly = np.convolve(poly, [1.0, 1.0])   # (1 + t)
        alpha += (comb(KBERN, k) / 2.0 ** KBERN) * float(theta[k]) * poly
    return alpha


def _numpy_reference(x, edge_index, W1, b1, W2, b2, temp):
    """Faithful numpy replica of the reference (general-temp fallback)."""
    n = x.shape[0]
    h = np.maximum(x @ W1 + b1, 0.0).astype(np.float32)
    h = (h @ W2 + b2).astype(np.float32)
    theta = np.maximum(temp.astype(np.float32), 0.0)
    row, col = edge_index[0], edge_index[1]
    deg = np.zeros(n, np.float32)
    np.add.at(deg, row, np.float32(1.0))
    dinv = np.where(deg > 0, 1.0 / np.sqrt(deg), 0.0).astype(np.float32)
    w = (dinv[row] * dinv[col])[:, None].astype(np.float32)

    def adj(v):
        out = np.zeros_like(v)
        np.add.at(out, row, v[col] * w)
        return out

    tmp = [h]
    v = h
    for _ in range(KBERN):
        v = v + adj(v)
        tmp.append(v)
    scale = np.float32(1.0 / 2.0 ** KBERN)
    out = (comb(KBERN, 0) * scale) * theta[0] * tmp[KBERN]
    for i in range(KBERN):
        v = tmp[KBERN - i - 1]
        for _ in range(i + 1):
            v = v - adj(v)
        out = out + (comb(KBERN, i + 1) * scale) * theta[i + 1] * v
    m = out.max(axis=1, keepdims=True)
    ex = np.exp(out - m)
    return ((out - m) - np.log(ex.sum(axis=1, keepdims=True))).astype(np.float32)


def kernel(**inputs) -> np.ndarray:
    x = np.ascontiguousarray(np.asarray(inputs["x"], dtype=np.float32))
    W1 = np.ascontiguousarray(np.asarray(inputs["W1"], dtype=np.float32))
    b1 = np.ascontiguousarray(np.asarray(inputs["b1"], dtype=np.float32))
    W2 = np.ascontiguousarray(np.asarray(inputs["W2"], dtype=np.float32))
    b2 = np.ascontiguousarray(np.asarray(inputs["b2"], dtype=np.float32))
    temp = np.asarray(inputs["temp"], dtype=np.float32)
    edge_index = np.asarray(inputs["edge_index"])

    theta = np.maximum(temp.astype(np.float64), 0.0)
    alpha = _bern_alpha(theta)
    collapses = abs(alpha[0] - 1.0) < 1e-9 and np.all(np.abs(alpha[1:]) < 1e-9)
    if not (collapses and x.shape == (N_NODES, F_IN) and W1.shape == (F_IN, F_MID)
            and W2.shape == (F_MID, F_OUT)):
        return _numpy_reference(x, edge_index.astype(np.int64), W1, b1, W2, b2, temp)

    n_pad = R_CORE * N_CORES
    xp = np.zeros((n_pad, F_IN), np.float32)
    xp[:N_NODES] = x
    in_maps = [
        {
            "x": np.ascontiguousarray(xp[i * R_CORE:(i + 1) * R_CORE]),
            "W1": W1, "b1": b1, "W2": W2, "b2": b2,
        }
        for i in range(N_CORES)
    ]
    nc = _build_program()
    res = run_bass_kernel_spmd(nc, in_maps, list(range(N_CORES))).results
    out = np.concatenate([res[i]["out"] for i in range(N_CORES)], axis=0)
    return np.ascontiguousarray(out[:N_NODES])
